# revision 20
# baseline (speedup 1.0000x reference)
"""BPaCo+ loss on 8 TRN2 NeuronCores.

Two-consumer std-layout pipeline. Each core owns K/8 = 4096 queue columns;
per anchor-iblock phase, fp8 DoubleRow matmuls (lnr(j) on contraction row
128) fill two PSUM tiles: SP [128, 3072] consumed by the scalar engine (one
split Exp pair per phase, softmax partials via the activation accumulator)
and SD [128, 1024] consumed by the vector engine (Schraudolph exp: int16 =
A*x + B gives the bits of bf16(e^x), then a bf16 tensor_reduce).

The TRN2 PE DVFS drops to 1.2 GHz on any semaphore wait and only ramps on
long uninterrupted plain-matmul chains, which a consumer-paced pipeline
can't provide; paying the ~5us ramp with junk matmuls costs as much as the
boost buys. So this kernel is deliberately PE-paced at the sustained clock:
64 back-to-back matmuls per core with every consumer dependency satisfied
ahead of arrival (split exps release SP banks early; SD is double-phase
buffered by the ts instruction finishing within a phase).

Host computes the O(B*(B+C)) blocks exactly; the vector-engine sums carry a
small systematic Schraudolph bias divided out on host (BETA_CAL).
"""
import numpy as np
import ml_dtypes

from concourse import bass, bacc, mybir, tile
from concourse.bass_utils import run_bass_kernel_spmd

B, K, C, D = 1024, 32768, 100, 128
T, ALPHA = 0.07, 0.05
M = 8                       # cores
QSH = K // M                # 4096 queue cols per core
QS = 2560                   # scalar-engine queue cols per iblock (a+b)
QD = QSH - QS               # 1536 vector-engine queue cols per iblock
IB = 8                      # anchor i-blocks of 128

BF16 = mybir.dt.bfloat16
F32 = mybir.dt.float32
FP8 = mybir.dt.float8e4
I16 = mybir.dt.int16
NP_FP8 = ml_dtypes.float8_e4m3

# Schraudolph constants for bf16 bit pattern: bits = A1*x + B1OFF
A1 = float(1 << 7) / np.log(2.0)
B1OFF = 127.0 * (1 << 7) + 0.5 - 8.0
# systematic multiplicative bias of the bf16 Schraudolph exp; vector-engine
# sums are divided by (1 + BETA_CAL) on host
BETA_CAL = 0.0

_CACHE = {}


def _build_nc():
    nc = bacc.Bacc(None, target_bir_lowering=False)
    # anchors DR-packed (fTq + ones row) cols 0:512 fused with the first
    # queue piece so one DMA (and one semaphore) gates the first matmul
    XA = nc.declare_dram_parameter("XA", [65, 2048], FP8, isOutput=False)
    X1 = nc.declare_dram_parameter("X1", [65, 1024], FP8, isOutput=False)
    # queue DR-packed with lnr row [65, 2, 4096], split for DMA pipelining
    RQ1 = nc.declare_dram_parameter("RQ1", [65, 2048], FP8, isOutput=False)
    RQ2 = nc.declare_dram_parameter("RQ2", [65, 2048], FP8, isOutput=False)
    RQ3 = nc.declare_dram_parameter("RQ3", [65, 2048], FP8, isOutput=False)
    RQ4 = nc.declare_dram_parameter("RQ4", [65, 1024], FP8, isOutput=False)
    ACC = nc.declare_dram_parameter("ACC", [128, 32], F32, isOutput=True)

    with tile.TileContext(nc) as tc:
        with (
            tc.tile_pool(name="sb", bufs=1) as sbp,
            tc.tile_pool(name="ps", bufs=1, space=bass.MemorySpace.PSUM) as pps,
        ):
            XA_sb = sbp.tile([65, 2, 1024], FP8, tag="XA")
            X1_sb = sbp.tile([65, 2, 512], FP8, tag="X1")
            RA1_sb = sbp.tile([65, 2, 1024], FP8, tag="RA1")
            RB_sb = sbp.tile([65, 2, 1024], FP8, tag="RB")
            RD0_sb = sbp.tile([65, 2, 1024], FP8, tag="RD0")
            RD1_sb = sbp.tile([65, 2, 512], FP8, tag="RD1")

            # arrival order matches first-phase consumption: SPa cols
            # first, then SPb cols, then SD cols
            # early-critical data rides the sync hwdge queue (the scalar
            # queue opens with the 1.3us exp-table load, and gpsimd SWDGE
            # completion signals ~3us late behind a queue drain)
            nc.sync.dma_start(XA_sb[:], XA[:])
            nc.sync.dma_start(RD0_sb[:], RQ3[:])
            nc.sync.dma_start(RD1_sb[:], RQ4[:])
            nc.scalar.dma_start(RA1_sb[:], RQ1[:])
            nc.scalar.dma_start(RB_sb[:], RQ2[:])
            nc.gpsimd.dma_start(X1_sb[:], X1[:])

            # ACC cols per phase: i1, i2 accums + 2 vector reduces
            ACC_sb = sbp.tile([128, 32], F32, tag="ACCsb")
            Etrash = sbp.tile([128, QS], BF16, tag="Etrash")

            warm = sbp.tile([128, 1], F32, tag="warm")
            nc.gpsimd.memset(warm[:], 0.0)
            nc.scalar.activation(
                warm[:], warm[:], mybir.ActivationFunctionType.Exp)

            SPa = pps.tile([128, 1536], F32, tag="SPa")   # 3 banks
            SPb = pps.tile([128, 1024], F32, tag="SPb")   # 2 banks
            SD = pps.tile([128, QD], F32, tag="SD")       # 3 banks

            def xblk(b):
                if b < 4:
                    return XA_sb[:, :, b * 128:b * 128 + 128]
                return X1_sb[:, :, (b - 4) * 128:(b - 4) * 128 + 128]

            DR = mybir.MatmulPerfMode.DoubleRow
            for p in range(IB):
                # separate PSUM tiles per consumer instruction keep the
                # semaphore thresholds tile-exact; tile_wait_until pins the
                # scheduler to phase order
                if True:
                    a_srcs = (XA_sb[:, :, 512:1024], RA1_sb[:, :, 0:512],
                              RA1_sb[:, :, 512:1024])
                    d_srcs = (RD0_sb[:, :, 0:512], RD0_sb[:, :, 512:1024],
                              RD1_sb[:])
                    b_srcs = (RB_sb[:, :, 0:512], RB_sb[:, :, 512:1024])
                    dly = 0.0013 if p else 0.0024
                    if p == IB - 1:
                        # last phase leads with the vector-engine chunks so
                        # the ts/fold/reduce tail starts as early as possible
                        with tc.tile_wait_until(p * 0.0034):
                            for k in range(3):
                                nc.tensor.matmul(
                                    SD[:, k * 512:k * 512 + 512], xblk(p),
                                    d_srcs[k],
                                    start=True, stop=True, perf_mode=DR,
                                )
                        with tc.tile_wait_until(p * 0.0034 + 0.0013):
                            for k in range(3):
                                nc.tensor.matmul(
                                    SPa[:, k * 512:k * 512 + 512], xblk(p),
                                    a_srcs[k],
                                    start=True, stop=True, perf_mode=DR,
                                )
                    else:
                        with tc.tile_wait_until(p * 0.0034):
                            for k in range(3):
                                nc.tensor.matmul(
                                    SPa[:, k * 512:k * 512 + 512], xblk(p),
                                    a_srcs[k],
                                    start=True, stop=True, perf_mode=DR,
                                )
                        with tc.tile_wait_until(p * 0.0034 + dly):
                            for k in range(3):
                                nc.tensor.matmul(
                                    SD[:, k * 512:k * 512 + 512], xblk(p),
                                    d_srcs[k],
                                    start=True, stop=True, perf_mode=DR,
                                )
                    with tc.tile_wait_until(p * 0.0034 + dly + 0.0013):
                        for k in range(2):
                            nc.tensor.matmul(
                                SPb[:, k * 512:k * 512 + 512], xblk(p),
                                b_srcs[k],
                                start=True, stop=True, perf_mode=DR,
                            )
                    nc.scalar.activation(
                        Etrash[:, 0:1536], SPa[:],
                        mybir.ActivationFunctionType.Exp,
                        accum_out=ACC_sb[:, 4 * p:4 * p + 1],
                    )
                    e16 = sbp.tile([128, QD], I16, tag="E", bufs=2)
                    nc.vector.tensor_scalar(
                        e16[:], SD[:], A1, B1OFF,
                        mybir.AluOpType.mult, mybir.AluOpType.add,
                    )
                    # bf16 pair-fold halves the reduce cost (tensor_tensor
                    # runs 2x on bf16, tensor_reduce does not)
                    ebf = e16[:].bitcast(BF16)
                    f1 = sbp.tile([128, 768], BF16, tag="F1", bufs=2)
                    nc.vector.tensor_tensor(
                        f1[:], ebf[:, 0:768], ebf[:, 768:1536],
                        mybir.AluOpType.add)
                    f2 = sbp.tile([128, 384], BF16, tag="F2", bufs=2)
                    nc.vector.tensor_tensor(
                        f2[:], f1[:, 0:384], f1[:, 384:768],
                        mybir.AluOpType.add)
                    nc.vector.tensor_reduce(
                        ACC_sb[:, 4 * p + 2:4 * p + 3], f2[:],
                        axis=mybir.AxisListType.X, op=mybir.AluOpType.add,
                    )
                    nc.scalar.activation(
                        Etrash[:, 1536:QS], SPb[:],
                        mybir.ActivationFunctionType.Exp,
                        accum_out=ACC_sb[:, 4 * p + 1:4 * p + 2],
                    )

            nc.sync.dma_start(ACC[:], ACC_sb[:])

    nc.compile()
    return nc


def _prep_inputs(features, labels):
    f = features.astype(np.float64)
    lab = labels.astype(np.int64)
    ccount = np.bincount(lab, minlength=C).astype(np.float64)

    lnr0 = -np.log(ccount)
    s2 = -np.median(lnr0[lab])
    lnr0p = lnr0[lab] + s2

    fq = f.astype(NP_FP8).astype(np.float32)
    fTq = (f[:B] / T).astype(NP_FP8).astype(np.float32)

    lx = np.zeros((130, B), np.float32)
    lx[:D] = fTq.T
    lx[D] = 1.0
    X = np.ascontiguousarray(lx.reshape(65, 2, B)).astype(NP_FP8)
    X0 = np.ascontiguousarray(X[:, :, 0:512])          # [65, 2, 512]
    X1 = np.ascontiguousarray(X[:, :, 512:1024]).reshape(65, -1)

    in_maps = []
    for c in range(M):
        jQ = slice(B + c * QSH, B + (c + 1) * QSH)
        rq = np.zeros((130, QSH), np.float32)
        rq[:D] = fq[jQ].T
        rq[D] = lnr0p[jQ]
        rq = rq.reshape(65, 2, QSH).astype(NP_FP8)
        im = {"X1": X1}
        im["XA"] = np.ascontiguousarray(
            np.concatenate([X0, rq[:, :, 0:512]], axis=2)).reshape(65, -1)
        for name, a, b in (
            ("RQ1", 512, 1536), ("RQ2", 1536, 2560),
            ("RQ3", 2560, 3584), ("RQ4", 3584, 4096),
        ):
            im[name] = np.ascontiguousarray(rq[:, :, a:b]).reshape(65, -1)
        in_maps.append(im)
    return in_maps, s2


def kernel(features, sup_logits, centers, labels, _debug=False, _trace=False):
    if "nc" not in _CACHE:
        _CACHE["nc"] = _build_nc()
    nc = _CACHE["nc"]
    in_maps, s2 = _prep_inputs(features, labels)
    res = run_bass_kernel_spmd(nc, in_maps, core_ids=list(range(M)), trace=_trace)
    _CACHE["last"] = res

    S2q = np.zeros(B, np.float64)
    for c in range(M):
        acc = res.results[c]["ACC"].astype(np.float64)     # [128, 32]
        act_part = acc[:, 0::4] + acc[:, 1::4]             # [128, 8]
        dve_part = acc[:, 2::4] / (1.0 + BETA_CAL)         # [128, 8]
        S2q += (act_part + dve_part).T.reshape(B)
    S2q *= np.exp(-s2)

    # ---- host blocks (exact): batch-vs-batch, branch 1, sup logits ----
    f = features.astype(np.float64)
    f32b = features.astype(np.float32)
    sup = sup_logits.astype(np.float64)
    lab = labels.astype(np.int64)
    labB = lab[:B]
    ccount = np.bincount(lab, minlength=C).astype(np.float64)
    cntB = np.bincount(labB, minlength=C).astype(np.float64)
    cc1 = cntB + 1.0

    cols = np.concatenate([f32b[:B], centers.astype(np.float32)], axis=0)
    LG = (f32b[:B] @ cols.T) / np.float32(T)          # [B, B+C]
    ELG = np.exp(LG.astype(np.float64))
    ELG[np.arange(B), np.arange(B)] = 0.0             # diag masked in both branches

    match_bb = labB[:, None] == labB[None, :]
    W2 = 1.0 / (ccount[labB][None, :] - ALPHA * match_bb)
    S2h = (ELG[:, :B] * W2).sum(1)
    oh = labB[:, None] == np.arange(C)[None, :]
    S2sup = (np.exp(sup) / (ccount[None, :] - oh)).sum(1)
    S2 = S2q + S2h + S2sup

    lab1 = np.concatenate([labB, np.arange(C)])
    match1 = labB[:, None] == lab1[None, :]
    W1 = 1.0 / (cc1[lab1][None, :] - match1)  # diag already zeroed in ELG
    S1 = (ELG * W1).sum(1)

    g2 = np.zeros((C, D))
    np.add.at(g2, lab, f)
    g1 = np.zeros((C, D))
    np.add.at(g1, labB, f[:B])
    g1 += centers.astype(np.float64)
    A2 = np.einsum("id,id->i", f[:B], g2[labB]) / T - 1.0 / T
    A1h = np.einsum("id,id->i", f[:B], g1[labB]) / T - 1.0 / T

    msum = 1.0 + ALPHA * (ccount[labB] - 1.0)
    numer2 = sup[np.arange(B), labB] + ALPHA * A2
    loss2 = np.mean(np.log(S2) - numer2 / msum)
    loss1 = np.mean(np.log(S1) - A1h / cntB[labB])
    return np.array(loss1 + loss2, dtype=np.float32)


# revision 21
# speedup vs baseline: 1.1703x; 1.1703x over previous
"""BPaCo+ loss on 8 TRN2 NeuronCores.

Two-consumer std-layout pipeline. Each core owns K/8 = 4096 queue columns;
per anchor-iblock phase, fp8 DoubleRow matmuls (lnr(j) on contraction row
128) fill two PSUM tiles: SP [128, 3072] consumed by the scalar engine (one
split Exp pair per phase, softmax partials via the activation accumulator)
and SD [128, 1024] consumed by the vector engine (Schraudolph exp: int16 =
A*x + B gives the bits of bf16(e^x), then a bf16 tensor_reduce).

The TRN2 PE DVFS drops to 1.2 GHz on any semaphore wait and only ramps on
long uninterrupted plain-matmul chains, which a consumer-paced pipeline
can't provide; paying the ~5us ramp with junk matmuls costs as much as the
boost buys. So this kernel is deliberately PE-paced at the sustained clock:
64 back-to-back matmuls per core with every consumer dependency satisfied
ahead of arrival (split exps release SP banks early; SD is double-phase
buffered by the ts instruction finishing within a phase).

Host computes the O(B*(B+C)) blocks exactly; the vector-engine sums carry a
small systematic Schraudolph bias divided out on host (BETA_CAL).
"""
import numpy as np
import ml_dtypes

from concourse import bass, bacc, mybir, tile
from concourse.bass_utils import run_bass_kernel_spmd

B, K, C, D = 1024, 32768, 100, 128
T, ALPHA = 0.07, 0.05
M = 8                       # cores
QSH = K // M                # 4096 queue cols per core
QS = 2560                   # scalar-engine queue cols per iblock (a+b)
QD = QSH - QS               # 1536 vector-engine queue cols per iblock
IB = 8                      # anchor i-blocks of 128

BF16 = mybir.dt.bfloat16
F32 = mybir.dt.float32
FP8 = mybir.dt.float8e4
I16 = mybir.dt.int16
NP_FP8 = ml_dtypes.float8_e4m3

# Schraudolph constants for bf16 bit pattern: bits = A1*x + B1OFF
A1 = float(1 << 7) / np.log(2.0)
B1OFF = 127.0 * (1 << 7) + 0.5 - 8.0
# systematic multiplicative bias of the bf16 Schraudolph exp; vector-engine
# sums are divided by (1 + BETA_CAL) on host
BETA_CAL = 0.0

_CACHE = {}


def _build_nc():
    nc = bacc.Bacc(None, target_bir_lowering=False)
    # anchors DR-packed (fTq + ones row) cols 0:512 fused with the first
    # queue piece so one DMA (and one semaphore) gates the first matmul
    XA = nc.declare_dram_parameter("XA", [65, 2048], FP8, isOutput=False)
    X1 = nc.declare_dram_parameter("X1", [65, 1024], FP8, isOutput=False)
    # queue DR-packed with lnr row [65, 2, 4096], split for DMA pipelining
    RQ1 = nc.declare_dram_parameter("RQ1", [65, 2048], FP8, isOutput=False)
    RQ2 = nc.declare_dram_parameter("RQ2", [65, 2048], FP8, isOutput=False)
    RQ3 = nc.declare_dram_parameter("RQ3", [65, 2048], FP8, isOutput=False)
    RQ4 = nc.declare_dram_parameter("RQ4", [65, 1024], FP8, isOutput=False)
    ACC = nc.declare_dram_parameter("ACC", [128, 32], F32, isOutput=True)

    with tile.TileContext(nc) as tc:
        with (
            tc.tile_pool(name="sb", bufs=1) as sbp,
            tc.tile_pool(name="ps", bufs=1, space=bass.MemorySpace.PSUM) as pps,
        ):
            XA_sb = sbp.tile([65, 2, 1024], FP8, tag="XA")
            X1_sb = sbp.tile([65, 2, 512], FP8, tag="X1")
            RA1_sb = sbp.tile([65, 2, 1024], FP8, tag="RA1")
            RB_sb = sbp.tile([65, 2, 1024], FP8, tag="RB")
            RD0_sb = sbp.tile([65, 2, 1024], FP8, tag="RD0")
            RD1_sb = sbp.tile([65, 2, 512], FP8, tag="RD1")

            # arrival order matches first-phase consumption: SPa cols
            # first, then SPb cols, then SD cols
            # early-critical data rides the sync hwdge queue (the scalar
            # queue opens with the 1.3us exp-table load, and gpsimd SWDGE
            # completion signals ~3us late behind a queue drain)
            nc.sync.dma_start(XA_sb[:], XA[:])
            nc.sync.dma_start(RD0_sb[:], RQ3[:])
            nc.sync.dma_start(RD1_sb[:], RQ4[:])
            nc.scalar.dma_start(RA1_sb[:], RQ1[:])
            nc.scalar.dma_start(RB_sb[:], RQ2[:])
            nc.gpsimd.dma_start(X1_sb[:], X1[:])

            # ACC cols per phase: i1, i2 accums + 2 vector reduces
            ACC_sb = sbp.tile([128, 32], F32, tag="ACCsb")
            Etrash = sbp.tile([128, QS], BF16, tag="Etrash")

            warm = sbp.tile([128, 1], F32, tag="warm")
            nc.gpsimd.memset(warm[:], 0.0)
            nc.scalar.activation(
                warm[:], warm[:], mybir.ActivationFunctionType.Exp)

            SPa = pps.tile([128, 1536], F32, tag="SPa")   # 3 banks
            SPb = pps.tile([128, 1024], F32, tag="SPb")   # 2 banks
            SD = pps.tile([128, QD], F32, tag="SD")       # 3 banks

            def xblk(b):
                if b < 4:
                    return XA_sb[:, :, b * 128:b * 128 + 128]
                return X1_sb[:, :, (b - 4) * 128:(b - 4) * 128 + 128]

            DR = mybir.MatmulPerfMode.DoubleRow
            for p in range(IB):
                # separate PSUM tiles per consumer instruction keep the
                # semaphore thresholds tile-exact; tile_wait_until pins the
                # scheduler to phase order
                if True:
                    a_srcs = (XA_sb[:, :, 512:1024], RA1_sb[:, :, 0:512],
                              RA1_sb[:, :, 512:1024])
                    d_srcs = (RD0_sb[:, :, 0:512], RD0_sb[:, :, 512:1024],
                              RD1_sb[:])
                    b_srcs = (RB_sb[:, :, 0:512], RB_sb[:, :, 512:1024])
                    dly = 0.0013 if p else 0.0024
                    with tc.tile_wait_until(p * 0.0034):
                        for k in range(3):
                            nc.tensor.matmul(
                                SPa[:, k * 512:k * 512 + 512], xblk(p),
                                a_srcs[k],
                                start=True, stop=True, perf_mode=DR,
                            )
                    with tc.tile_wait_until(p * 0.0034 + dly):
                        for k in range(3):
                            nc.tensor.matmul(
                                SD[:, k * 512:k * 512 + 512], xblk(p),
                                d_srcs[k],
                                start=True, stop=True, perf_mode=DR,
                            )
                    with tc.tile_wait_until(p * 0.0034 + dly + 0.0013):
                        for k in range(2):
                            nc.tensor.matmul(
                                SPb[:, k * 512:k * 512 + 512], xblk(p),
                                b_srcs[k],
                                start=True, stop=True, perf_mode=DR,
                            )
                    nc.scalar.activation(
                        Etrash[:, 0:1536], SPa[:],
                        mybir.ActivationFunctionType.Exp,
                        accum_out=ACC_sb[:, 4 * p:4 * p + 1],
                    )
                    e16 = sbp.tile([128, QD], I16, tag="E", bufs=2)
                    nc.vector.tensor_scalar(
                        e16[:], SD[:], A1, B1OFF,
                        mybir.AluOpType.mult, mybir.AluOpType.add,
                    )
                    # bf16 pair-fold halves the reduce cost (tensor_tensor
                    # runs 2x on bf16, tensor_reduce does not)
                    ebf = e16[:].bitcast(BF16)
                    f1 = sbp.tile([128, 768], BF16, tag="F1", bufs=2)
                    nc.vector.tensor_tensor(
                        f1[:], ebf[:, 0:768], ebf[:, 768:1536],
                        mybir.AluOpType.add)
                    f2 = sbp.tile([128, 384], BF16, tag="F2", bufs=2)
                    nc.vector.tensor_tensor(
                        f2[:], f1[:, 0:384], f1[:, 384:768],
                        mybir.AluOpType.add)
                    nc.vector.tensor_reduce(
                        ACC_sb[:, 4 * p + 2:4 * p + 3], f2[:],
                        axis=mybir.AxisListType.X, op=mybir.AluOpType.add,
                    )
                    nc.scalar.activation(
                        Etrash[:, 1536:QS], SPb[:],
                        mybir.ActivationFunctionType.Exp,
                        accum_out=ACC_sb[:, 4 * p + 1:4 * p + 2],
                    )

            nc.sync.dma_start(ACC[:], ACC_sb[:])

    nc.compile()
    return nc


def _prep_inputs(features, labels):
    f = features.astype(np.float64)
    lab = labels.astype(np.int64)
    ccount = np.bincount(lab, minlength=C).astype(np.float64)

    lnr0 = -np.log(ccount)
    s2 = -np.median(lnr0[lab])
    lnr0p = lnr0[lab] + s2

    fq = f.astype(NP_FP8).astype(np.float32)
    fTq = (f[:B] / T).astype(NP_FP8).astype(np.float32)

    lx = np.zeros((130, B), np.float32)
    lx[:D] = fTq.T
    lx[D] = 1.0
    X = np.ascontiguousarray(lx.reshape(65, 2, B)).astype(NP_FP8)
    X0 = np.ascontiguousarray(X[:, :, 0:512])          # [65, 2, 512]
    X1 = np.ascontiguousarray(X[:, :, 512:1024]).reshape(65, -1)

    in_maps = []
    for c in range(M):
        jQ = slice(B + c * QSH, B + (c + 1) * QSH)
        rq = np.zeros((130, QSH), np.float32)
        rq[:D] = fq[jQ].T
        rq[D] = lnr0p[jQ]
        rq = rq.reshape(65, 2, QSH).astype(NP_FP8)
        im = {"X1": X1}
        im["XA"] = np.ascontiguousarray(
            np.concatenate([X0, rq[:, :, 0:512]], axis=2)).reshape(65, -1)
        for name, a, b in (
            ("RQ1", 512, 1536), ("RQ2", 1536, 2560),
            ("RQ3", 2560, 3584), ("RQ4", 3584, 4096),
        ):
            im[name] = np.ascontiguousarray(rq[:, :, a:b]).reshape(65, -1)
        in_maps.append(im)
    return in_maps, s2


def kernel(features, sup_logits, centers, labels, _debug=False, _trace=False):
    if "nc" not in _CACHE:
        _CACHE["nc"] = _build_nc()
    nc = _CACHE["nc"]
    in_maps, s2 = _prep_inputs(features, labels)
    res = run_bass_kernel_spmd(nc, in_maps, core_ids=list(range(M)), trace=_trace)
    _CACHE["last"] = res

    S2q = np.zeros(B, np.float64)
    for c in range(M):
        acc = res.results[c]["ACC"].astype(np.float64)     # [128, 32]
        act_part = acc[:, 0::4] + acc[:, 1::4]             # [128, 8]
        dve_part = acc[:, 2::4] / (1.0 + BETA_CAL)         # [128, 8]
        S2q += (act_part + dve_part).T.reshape(B)
    S2q *= np.exp(-s2)

    # ---- host blocks (exact): batch-vs-batch, branch 1, sup logits ----
    f = features.astype(np.float64)
    f32b = features.astype(np.float32)
    sup = sup_logits.astype(np.float64)
    lab = labels.astype(np.int64)
    labB = lab[:B]
    ccount = np.bincount(lab, minlength=C).astype(np.float64)
    cntB = np.bincount(labB, minlength=C).astype(np.float64)
    cc1 = cntB + 1.0

    cols = np.concatenate([f32b[:B], centers.astype(np.float32)], axis=0)
    LG = (f32b[:B] @ cols.T) / np.float32(T)          # [B, B+C]
    ELG = np.exp(LG.astype(np.float64))
    ELG[np.arange(B), np.arange(B)] = 0.0             # diag masked in both branches

    match_bb = labB[:, None] == labB[None, :]
    W2 = 1.0 / (ccount[labB][None, :] - ALPHA * match_bb)
    S2h = (ELG[:, :B] * W2).sum(1)
    oh = labB[:, None] == np.arange(C)[None, :]
    S2sup = (np.exp(sup) / (ccount[None, :] - oh)).sum(1)
    S2 = S2q + S2h + S2sup

    lab1 = np.concatenate([labB, np.arange(C)])
    match1 = labB[:, None] == lab1[None, :]
    W1 = 1.0 / (cc1[lab1][None, :] - match1)  # diag already zeroed in ELG
    S1 = (ELG * W1).sum(1)

    g2 = np.zeros((C, D))
    np.add.at(g2, lab, f)
    g1 = np.zeros((C, D))
    np.add.at(g1, labB, f[:B])
    g1 += centers.astype(np.float64)
    A2 = np.einsum("id,id->i", f[:B], g2[labB]) / T - 1.0 / T
    A1h = np.einsum("id,id->i", f[:B], g1[labB]) / T - 1.0 / T

    msum = 1.0 + ALPHA * (ccount[labB] - 1.0)
    numer2 = sup[np.arange(B), labB] + ALPHA * A2
    loss2 = np.mean(np.log(S2) - numer2 / msum)
    loss1 = np.mean(np.log(S1) - A1h / cntB[labB])
    return np.array(loss1 + loss2, dtype=np.float32)
